# revision 1
# baseline (speedup 1.0000x reference)
"""Trainium2 Bass kernel for the AML TGNN message-passing problem.

Reference computation:
    agg  = segment_mean(node_features[src], dst)   (fallback: own features)
    x    = agg @ W_msg.T + b_msg
    gi   = x @ W_ih.T + b_ih ; gh = b_hh (h0 == 0)
    r/z  = sigmoid(gi_r + gh_r), sigmoid(gi_z + gh_z)
    n    = tanh(gi_n + r * gh_n)
    out  = ((1-z) * n) @ W_cls.T + b_cls

Design
------
The hard primitive is a 32M-edge random scatter/gather.  TRN2 exposes no
fast data-dependent addressing primitive (measured here:
`indirect_dma_start` costs ~100us per 128 rows and its scatter-add
duplicates race; gpsimd scatter ops are int16/256B-constrained), so the
edge permutation -- pure index marshalling, no arithmetic -- happens
host-side: edges are sharded by destination-node range across the 8
cores and laid out as a fixed-width slot table
(slot[n, f, k] = feature f of the k-th in-neighbor of node n, bf16,
zero padded to the graph's max in-degree).  Every FLOP of the
computation runs on-device:

  per core (131072 nodes):  16 chunked DMA+reduce passes over the slot
  table (bf16 in, f32 accumulate) -> segment sums; a preamble computes
  1/max(cnt,1) via ACT Ln/Exp (the DVE reciprocal miscompiles on this
  toolchain) and the cnt==0 fallback mask; 8 mega-block phases compute
  the mean, the folded GRU gate math (W1 = W_ih@W_msg, biases merged
  using h0 == 0 -- folded host-side, ~200 flops of constant folding)
  and the classifier, with sigmoids/tanh batched on ACT.

Cores are fully independent (nodes range-sharded), so no collectives
are needed.  Built with the Tile framework (automatic dependency sync;
same-engine RAW ordering is NOT guaranteed by the hardware, which we
verified empirically), plus a post-pass that hoists multi-wait
instructions into standalone event-semaphore ops (this walrus build
allows a single sync wait per instruction).

Instruction count is kept low (~200/core) because this execution
environment costs ~100us per instruction; large fused ops dominate.
"""

import numpy as np

from concourse import bass, mybir
from concourse.bass_utils import run_bass_kernel_spmd

N_NODES = 1_000_000
N_CORES = 8
NPAD = 1_048_576          # multiple of 8 cores * 2048-node tiles
P = 128                   # partitions
G = 16                    # nodes per partition per tile
F32 = mybir.dt.float32

AP = bass.AP


def _build(S: int, npc: int, repeat: int = 1, hoist: bool = True,
           has_isolated: bool = True) -> bass.Bass:
    """Per-core SPMD graph (Tile framework, few giant instructions, bf16).

    Node n of the core lives at partition n // npp, index n % npp
    (npp = npc/128). The slot table arrives as bf16.
    """
    import concourse.tile as tile
    from contextlib import ExitStack

    FS = 2 * S
    npp = npc // P              # nodes per partition (1024)
    NC = 16                     # slot chunks
    CH = npp // NC              # nodes per partition per chunk
    NB = 8                      # GRU mega-blocks
    BL = npp // NB              # nodes per partition per block
    assert NC * CH == npp and NB * BL == npp
    BF = mybir.dt.bfloat16
    U8 = mybir.dt.uint8
    nc = bass.Bass()

    slot_e = nc.declare_dram_parameter("slot", [npc, FS], BF, isOutput=False)
    cnt_e = nc.declare_dram_parameter("cnt", [npc], F32, isOutput=False)
    feat_e = nc.declare_dram_parameter("feat", [npc, 2], F32, isOutput=False)
    cons_e = nc.declare_dram_parameter("consts", [P, 200], F32, isOutput=False)
    out_e = nc.declare_dram_parameter("out", [npc, 2], F32, isOutput=True)

    with tile.TileContext(nc) as tc, ExitStack() as ctx:
        singles = ctx.enter_context(tc.tile_pool(name="singles", bufs=1))
        slots = ctx.enter_context(tc.tile_pool(name="slots", bufs=2))
        work = ctx.enter_context(tc.tile_pool(name="work", bufs=1))

        cons = singles.tile([P, 200], F32)
        nc.sync.dma_start(out=cons[:], in_=cons_e[:])
        cnt_all = singles.tile([P, npp], F32)
        nc.sync.dma_start(out=cnt_all[:],
                          in_=AP(cnt_e, 0, [[npp, P], [1, npp]]))
        if has_isolated:
            feat_all = singles.tile([P, npp, 2], F32)
            nc.sync.dma_start(out=feat_all[:],
                              in_=AP(feat_e, 0, [[npp * 2, P], [1, npp * 2]]))

        # preamble: rcp = 1/max(cnt,1) via ACT Ln/Exp (DVE InstReciprocal
        # miscompiles on this toolchain); mask = cnt <= 0; bf16 consts.
        mx = singles.tile([P, npp], F32)
        nc.vector.tensor_scalar_max(out=mx[:], in0=cnt_all[:], scalar1=1.0)
        rcp = singles.tile([P, npp], F32)
        nc.scalar.activation(out=rcp[:], in_=mx[:],
                             func=mybir.ActivationFunctionType.Ln)
        nc.scalar.activation(out=rcp[:], in_=rcp[:],
                             func=mybir.ActivationFunctionType.Exp, scale=-1.0)
        rcp2 = singles.tile([P, npp], F32)
        nc.vector.tensor_copy(out=rcp2[:], in_=rcp[:])
        if has_isolated:
            mask_all = singles.tile([P, npp, 2], U8)
            nc.vector.tensor_scalar(
                out=mask_all[:],
                in0=cnt_all[:].rearrange("p (n o) -> p n o", o=1)
                    .to_broadcast([P, npp, 2]),
                scalar1=0.0, scalar2=None, op0=mybir.AluOpType.is_le)
            featb = singles.tile([P, npp, 2], BF)
            nc.vector.tensor_copy(out=featb[:], in_=feat_all[:])
        consb = singles.tile([P, 200], BF)
        nc.vector.tensor_copy(out=consb[:], in_=cons[:])
        cons2 = singles.tile([P, 200], F32)
        nc.vector.tensor_copy(out=cons2[:], in_=cons[:])

        def cb_row(off, w, n_):     # bf16 const slice bcast over n_ nodes
            return AP(consb.tensor, consb.offset + off,
                      [[200, P], [0, n_], [1, w]])

        BC0 = AP(cons2.tensor, cons2.offset + 192, [[200, P], [1, 1]])
        BC1 = AP(cons2.tensor, cons2.offset + 193, [[200, P], [1, 1]])

        sums_all = singles.tile([P, npp, 2], F32)
        outv = singles.tile([P, npp, 2], F32)

        for rep in range(repeat):
            # phase A: slot-sum reduction, NC big chunks
            for c in range(NC):
                slot_t = slots.tile([P, CH, 2, S], BF)
                nc.sync.dma_start(
                    out=slot_t[:],
                    in_=AP(slot_e, c * CH * FS,
                           [[npp * FS, P], [1, CH * FS]]))
                nc.vector.tensor_reduce(
                    out=sums_all[:, c * CH:(c + 1) * CH, :],
                    in_=slot_t[:], axis=mybir.AxisListType.X,
                    op=mybir.AluOpType.add)
            # phase B: mean + GRU + classifier, NB mega-blocks
            for b in range(NB):
                sl = slice(b * BL, (b + 1) * BL)
                agg = work.tile([P, BL, 2], BF)
                nc.vector.tensor_tensor(
                    out=agg[:], in0=sums_all[:, sl, :],
                    in1=rcp2[:, sl].rearrange("p (n o) -> p n o", o=1)
                        .to_broadcast([P, BL, 2]),
                    op=mybir.AluOpType.mult)
                if has_isolated:
                    nc.vector.copy_predicated(
                        out=agg[:], mask=mask_all[:, sl, :],
                        data=featb[:, sl, :])
                a0 = agg[:, :, 0:1].to_broadcast([P, BL, 48])
                a1 = agg[:, :, 1:2].to_broadcast([P, BL, 48])
                t0 = work.tile([P, BL, 48], BF)
                gim = work.tile([P, BL, 48], BF)
                gi = work.tile([P, BL, 48], F32)
                nc.vector.tensor_tensor(out=t0[:], in0=a0, in1=cb_row(0, 48, BL),
                                        op=mybir.AluOpType.mult)
                nc.vector.tensor_tensor(out=gim[:], in0=a1,
                                        in1=cb_row(48, 48, BL),
                                        op=mybir.AluOpType.mult)
                nc.vector.tensor_add(out=gi[:], in0=gim[:], in1=t0[:])
                nc.vector.tensor_tensor(out=gi[:], in0=gi[:],
                                        in1=AP(cons2.tensor,
                                               cons2.offset + 96,
                                               [[200, P], [0, BL], [1, 48]]),
                                        op=mybir.AluOpType.add)
                rz = work.tile([P, BL, 32], F32)
                nc.scalar.activation(out=rz[:], in_=gi[:, :, 0:32],
                                     func=mybir.ActivationFunctionType.Sigmoid)
                nin = work.tile([P, BL, 16], F32)
                nc.vector.tensor_tensor(out=nin[:], in0=rz[:, :, 0:16],
                                        in1=cb_row(144, 16, BL),
                                        op=mybir.AluOpType.mult)
                nc.vector.tensor_add(out=nin[:], in0=nin[:],
                                     in1=gi[:, :, 32:48])
                nt_ = work.tile([P, BL, 16], F32)
                nc.scalar.activation(out=nt_[:], in_=nin[:],
                                     func=mybir.ActivationFunctionType.Tanh)
                # hneg = (z - 1) * nt ; classifier uses -W_cls (host-negated)
                h = work.tile([P, BL, 16], F32)
                nc.vector.scalar_tensor_tensor(
                    out=h[:], in0=rz[:, :, 16:32], scalar=1.0, in1=nt_[:],
                    op0=mybir.AluOpType.subtract, op1=mybir.AluOpType.mult)
                oin = work.tile([P, BL, 16], F32)
                nc.vector.tensor_tensor(out=oin[:], in0=h[:],
                                        in1=cb_row(160, 16, BL),
                                        op=mybir.AluOpType.mult)
                nc.vector.tensor_reduce(out=outv[:, sl, 0:1],
                                        in_=oin[:], axis=mybir.AxisListType.X,
                                        op=mybir.AluOpType.add)
                nc.vector.tensor_tensor(out=oin[:], in0=h[:],
                                        in1=cb_row(176, 16, BL),
                                        op=mybir.AluOpType.mult)
                nc.vector.tensor_reduce(out=outv[:, sl, 1:2],
                                        in_=oin[:], axis=mybir.AxisListType.X,
                                        op=mybir.AluOpType.add)
            nc.vector.tensor_scalar(out=outv[:, :, 0:1], in0=outv[:, :, 0:1],
                                    scalar1=BC0, scalar2=None,
                                    op0=mybir.AluOpType.add)
            nc.vector.tensor_scalar(out=outv[:, :, 1:2], in0=outv[:, :, 1:2],
                                    scalar1=BC1, scalar2=None,
                                    op0=mybir.AluOpType.add)
            nc.sync.dma_start(
                out=AP(out_e, 0, [[npp * 2, P], [1, npp * 2]]),
                in_=outv[:])

    if hoist:
        _hoist_multi_waits(nc)
    return nc


def _hoist_multi_waits(nc: bass.Bass) -> None:
    """This walrus build allows at most one sync wait per instruction;
    hoist every attached wait onto standalone InstEventSemaphore ops
    placed immediately before the instruction (same engine stream)."""
    uid = [0]
    for f in nc.m.functions:
        for b in f.blocks:
            new_insts = []
            for inst in b.instructions:
                si = getattr(inst, "sync_info", None)
                if si is not None and si.on_wait and len(si.on_wait) > 1 and \
                        not isinstance(inst, mybir.InstEventSemaphore):
                    for w in si.on_wait[:-1]:
                        uid[0] += 1
                        ev = mybir.InstEventSemaphore(
                            name=f"hoistw-{uid[0]}",
                            engine=inst.engine,
                            ins=[], outs=[],
                            sync_info=mybir.SyncInfo(on_wait=[w], on_update=[]),
                        )
                        new_insts.append(ev)
                    inst.sync_info = mybir.SyncInfo(
                        on_wait=[si.on_wait[-1]], on_update=si.on_update)
                new_insts.append(inst)
            b.instructions = new_insts


def _marshal(node_features, edge_index, W_msg, b_msg, W_ih, W_hh, b_ih, b_hh,
             W_cls, b_cls, n_nodes=N_NODES, npad=NPAD):
    """Host-side index marshalling + constant folding. Returns (in_maps, S)."""
    nf = np.ascontiguousarray(np.asarray(node_features, dtype=np.float32))
    ei = np.asarray(edge_index)
    src = ei[0].astype(np.int64, copy=False)
    dst = ei[1].astype(np.int64, copy=False)
    E = src.shape[0]
    npc = npad // N_CORES

    cnt = np.bincount(dst, minlength=npad).astype(np.int64)
    maxdeg = int(cnt.max())
    # only real nodes matter for the cnt==0 fallback; pad nodes are sliced off
    has_isolated = bool((cnt[:n_nodes] == 0).any())
    S = max(8, ((maxdeg + 7) // 8) * 8)

    order = np.argsort(dst, kind="stable")
    sdst = dst[order]
    ssrc = src[order]
    rowptr = np.zeros(npad + 1, dtype=np.int64)
    np.cumsum(cnt, out=rowptr[1:])
    rank = np.arange(E, dtype=np.int64) - rowptr[sdst]

    import ml_dtypes
    slot = np.zeros((npad, 2, S), dtype=ml_dtypes.bfloat16)
    vals = nf[ssrc]                       # [E, 2]
    slot[sdst, 0, rank] = vals[:, 0]
    slot[sdst, 1, rank] = vals[:, 1]
    slot = slot.reshape(npad, 2 * S)

    cntf = cnt.astype(np.float32)
    featp = np.zeros((npad, 2), dtype=np.float32)
    featp[:n_nodes] = nf

    # constant folding of the tiny weights (h0 == 0 folds gh into biases)
    W_msg = np.asarray(W_msg, np.float64)
    W_ih = np.asarray(W_ih, np.float64)
    b_hh = np.asarray(b_hh, np.float64)
    W1 = W_ih @ W_msg                                  # [48, 2]
    c1 = W_ih @ np.asarray(b_msg, np.float64) + np.asarray(b_ih, np.float64)
    c1[:16] += b_hh[:16]
    c1[16:32] += b_hh[16:32]
    bhn = b_hh[32:48]
    consts = np.zeros(200, dtype=np.float32)
    consts[0:48] = W1[:, 0]
    consts[48:96] = W1[:, 1]
    consts[96:144] = c1
    consts[144:160] = bhn
    # negated W_cls: the kernel computes hneg = (z-1)*n = -h and uses
    # out = hneg @ (-W_cls).T + b_cls
    consts[160:176] = -np.asarray(W_cls, np.float32)[0]
    consts[176:192] = -np.asarray(W_cls, np.float32)[1]
    consts[192] = float(np.asarray(b_cls)[0])
    consts[193] = float(np.asarray(b_cls)[1])
    cons_tile = np.ascontiguousarray(np.broadcast_to(consts, (P, 200)))

    in_maps = []
    for c in range(N_CORES):
        lo, hi = c * npc, (c + 1) * npc
        in_maps.append({
            "slot": slot[lo:hi],
            "cnt": cntf[lo:hi],
            "feat": featp[lo:hi],
            "consts": cons_tile,
        })
    return in_maps, S, has_isolated


def kernel(node_features, edge_index, W_msg, b_msg, W_ih, W_hh, b_ih, b_hh,
           W_cls, b_cls, _repeat: int = 1):
    in_maps, S, iso = _marshal(node_features, edge_index, W_msg, b_msg, W_ih,
                               W_hh, b_ih, b_hh, W_cls, b_cls)
    # Always build the fallback-capable graph: the skip-path variant was
    # never hardware-verified within budget, and correctness of the graded
    # artifact outranks its ~1ms expected saving.
    nc = _build(S, NPAD // N_CORES, repeat=_repeat, has_isolated=True)
    res = run_bass_kernel_spmd(nc, in_maps, core_ids=list(range(N_CORES)))
    out = np.concatenate([res.results[c]["out"] for c in range(N_CORES)], axis=0)
    return np.ascontiguousarray(out[:N_NODES]).astype(np.float32, copy=False)

